# revision 4
# baseline (speedup 1.0000x reference)
"""GRU message-passing kernel for 8 Trainium2 NeuronCores.

Sharding: data-parallel over the batch dim B=16 -> 2 images per core.
Layout: feature-major (h^T [F, R] per image) so all matmuls take
pre-transposed weights as the stationary operand and activations as the
moving operand -- no on-device transposes. Output transposed on host.
"""

import sys

if "/opt/trn_rl_repo" not in sys.path:
    sys.path.insert(0, "/opt/trn_rl_repo")

import numpy as np

import concourse.bass as bass
import concourse.mybir as mybir
import concourse.tile as tile
from concourse import bacc
from concourse.bass_utils import run_bass_kernel_spmd

B, R, F, I = 16, 1024, 1024, 1024
ITERS = 2
NCORES = 8
IMGS = B // NCORES  # images per core
P = 128
KT = F // P  # 8 k-tiles
MT = I // P  # 8 m-tiles
NB = 2  # column blocks of 512 (PSUM bank limit for fp32)
NBW = R // NB  # 512
INV_DENOM = 1.0 / float(R - 1)

F32 = mybir.dt.float32
F32R = mybir.dt.float32r




def build_program():
    nc = bacc.Bacc("TRN2", target_bir_lowering=False, debug=False, num_devices=NCORES)

    # ---- DRAM tensors (per-core inputs) ----
    # Feature-major initial h (= features^T): [img, kt, p, r]
    h0_d = nc.dram_tensor("h0", [IMGS, KT, P, R], F32R, kind="ExternalInput")
    # boxes^T with an appended ones-row (folds fc_box_b into the matmul):
    bx_d = nc.dram_tensor("bx", [IMGS, 5, R], F32R, kind="ExternalInput")
    # fc_box weights + bias as lhsT rows: [5, jt, q] (row 4 = fc_box_b)
    bw_d = nc.dram_tensor("bw", [5, KT, P], F32R, kind="ExternalInput")
    # fc_input_w^T tiles, per-m-tile contiguous: [mt, p(k), kt, q(m)]
    w1_d = nc.dram_tensor("w1", [MT, P, KT, P], F32R, kind="ExternalInput")
    # GRU weights^T grouped per output f-tile j: [j, kt, p(k), gate(3)*128]
    wih_d = nc.dram_tensor("wih", [KT, KT, P, 3 * P], F32R, kind="ExternalInput")
    whh_d = nc.dram_tensor("whh", [KT, KT, P, 3 * P], F32R, kind="ExternalInput")
    # biases, per-partition layout [p, tile]
    bi_d = nc.dram_tensor("bi", [P, MT], F32, kind="ExternalInput")  # fc_input_b
    brz_d = nc.dram_tensor("brz", [P, 2 * KT], F32, kind="ExternalInput")  # bih+bhh r,z
    bhn_d = nc.dram_tensor("bhn", [P, KT], F32, kind="ExternalInput")  # b_hh n
    bin_d = nc.dram_tensor("bin", [P, KT], F32, kind="ExternalInput")  # b_ih n
    out_d = nc.dram_tensor("out", [IMGS, KT, P, R], F32R, kind="ExternalOutput")

    with tile.TileContext(nc) as tc:
        with (
            tc.tile_pool(name="acts", bufs=1) as acts,
            tc.tile_pool(name="wpool", bufs=2) as wpool,
            tc.tile_pool(name="wg", bufs=3) as wgp,
            tc.tile_pool(name="small", bufs=1) as small,
            tc.tile_pool(name="tmp", bufs=2) as tmp,
            tc.tile_pool(name="stat", bufs=2) as stat,
            tc.tile_pool(name="pbig", bufs=2, space="PSUM") as pbig,
            tc.tile_pool(name="pgate", bufs=1, space="PSUM") as pgate,
        ):
            # persistent activations (per partition: 4 x 32KB = 128KB)
            hA = acts.tile([P, KT, R], F32R, tag="hA")
            hB = acts.tile([P, KT, R], F32R, tag="hB")
            aC = acts.tile([P, KT, R], F32R, tag="aC")
            xS = acts.tile([P, KT, R], F32R, tag="xS")

            # small constants

            bx_sb = small.tile([5, IMGS, R], F32R, tag="bx")
            bw_sb = small.tile([5, KT, P], F32R, tag="bw")
            bi_sb = small.tile([P, MT], F32, tag="bi")
            brz_sb = small.tile([P, 2 * KT], F32, tag="brz")
            bhn_sb = small.tile([P, KT], F32, tag="bhn")
            bin_sb = small.tile([P, KT], F32, tag="bin")
            nc.sync.dma_start(out=bx_sb, in_=bx_d.rearrange("i f r -> f i r"))
            nc.sync.dma_start(out=bw_sb, in_=bw_d[:])
            nc.sync.dma_start(out=bi_sb, in_=bi_d[:])
            nc.sync.dma_start(out=brz_sb, in_=brz_d[:])
            nc.sync.dma_start(out=bhn_sb, in_=bhn_d[:])
            nc.sync.dma_start(out=bin_sb, in_=bin_d[:])

            for img in range(IMGS):
                # load h0 for this image (feature-major)
                nc.sync.dma_start(
                    out=hA, in_=h0_d[img].rearrange("k p r -> p k r")
                )
                h_cur, h_new = hA, hB
                for it in range(ITERS):
                    # ---- phase BF+RELU: a = relu(h * box_feat) ----
                    for j in range(KT):
                        bf_ps = pbig.tile([P, R], F32, tag="big")
                        for nb in range(NB):
                            nc.tensor.matmul(
                                bf_ps[:, nb * NBW : (nb + 1) * NBW],
                                (bw_sb[:, j, :]),
                                (bx_sb[:, img, nb * NBW : (nb + 1) * NBW]),
                                start=True,
                                stop=True,
                            )
                        nc.vector.tensor_tensor(
                            aC[:, j, :], h_cur[:, j, :], bf_ps, mybir.AluOpType.mult
                        )
                        nc.scalar.activation(
                            out=aC[:, j, :],
                            in_=aC[:, j, :],
                            func=mybir.ActivationFunctionType.Relu,
                        )

                    # ---- phase X: x^T = W1 @ a, with fused row-sum ----
                    s1 = stat.tile([P, MT], F32, tag="s1")
                    for mt in range(MT):
                        w1_sb = wpool.tile([P, KT, P], F32R, tag="w1")
                        nc.sync.dma_start(out=w1_sb, in_=w1_d[mt])
                        x_ps = pbig.tile([P, R], F32, tag="big")
                        for nb in range(NB):
                            for k in range(KT):
                                nc.tensor.matmul(
                                    x_ps[:, nb * NBW : (nb + 1) * NBW],
                                    (w1_sb[:, k, :]),
                                    (aC[:, k, nb * NBW : (nb + 1) * NBW]),
                                    start=(k == 0),
                                    stop=(k == KT - 1),
                                )
                        ssum = stat.tile([P, 1], F32, tag="ssum")
                        nc.scalar.activation(
                            out=xS[:, mt, :],
                            in_=x_ps,
                            func=mybir.ActivationFunctionType.Identity,
                            accum_out=ssum,
                        )
                        # s1' = S/denom + fc_input_b
                        nc.scalar.activation(
                            out=s1[:, mt : mt + 1],
                            in_=ssum,
                            func=mybir.ActivationFunctionType.Identity,
                            bias=bi_sb[:, mt : mt + 1],
                            scale=INV_DENOM,
                        )

                    # ---- phase INP: inp = -x/denom + s1'  (in place) ----
                    for mt in range(MT):
                        nc.scalar.activation(
                            out=xS[:, mt, :],
                            in_=xS[:, mt, :],
                            func=mybir.ActivationFunctionType.Identity,
                            bias=s1[:, mt : mt + 1],
                            scale=-INV_DENOM,
                        )

                    # ---- phase GATES ----
                    for j in range(KT):
                        wih_sb = wgp.tile([P, KT, 3 * P], F32R, tag="wg")
                        nc.sync.dma_start(
                            out=wih_sb, in_=wih_d[j].rearrange("k p c -> p k c")
                        )
                        whh_sb = wgp.tile([P, KT, 3 * P], F32R, tag="wg")
                        nc.sync.dma_start(
                            out=whh_sb, in_=whh_d[j].rearrange("k p c -> p k c")
                        )
                        for nb in range(NB):
                            cs = slice(nb * NBW, (nb + 1) * NBW)
                            s_r = pgate.tile([P, NBW], F32, tag="s_r")
                            s_z = pgate.tile([P, NBW], F32, tag="s_z")
                            gi_n = pgate.tile([P, NBW], F32, tag="gi_n")
                            gh_n = pgate.tile([P, NBW], F32, tag="gh_n")
                            # s_r = sum_k wih_r^T inp + whh_r^T h
                            for k in range(KT):
                                nc.tensor.matmul(
                                    s_r,
                                    (wih_sb[:, k, 0:P]),
                                    (xS[:, k, cs]),
                                    start=(k == 0),
                                    stop=False,
                                )
                            for k in range(KT):
                                nc.tensor.matmul(
                                    s_r,
                                    (whh_sb[:, k, 0:P]),
                                    (h_cur[:, k, cs]),
                                    start=False,
                                    stop=(k == KT - 1),
                                )
                            for k in range(KT):
                                nc.tensor.matmul(
                                    s_z,
                                    (wih_sb[:, k, P : 2 * P]),
                                    (xS[:, k, cs]),
                                    start=(k == 0),
                                    stop=False,
                                )
                            for k in range(KT):
                                nc.tensor.matmul(
                                    s_z,
                                    (whh_sb[:, k, P : 2 * P]),
                                    (h_cur[:, k, cs]),
                                    start=False,
                                    stop=(k == KT - 1),
                                )
                            for k in range(KT):
                                nc.tensor.matmul(
                                    gi_n,
                                    (wih_sb[:, k, 2 * P : 3 * P]),
                                    (xS[:, k, cs]),
                                    start=(k == 0),
                                    stop=(k == KT - 1),
                                )
                            for k in range(KT):
                                nc.tensor.matmul(
                                    gh_n,
                                    (whh_sb[:, k, 2 * P : 3 * P]),
                                    (h_cur[:, k, cs]),
                                    start=(k == 0),
                                    stop=(k == KT - 1),
                                )
                            # elementwise gates
                            r_t = tmp.tile([P, NBW], F32, tag="r_t")
                            z_t = tmp.tile([P, NBW], F32, tag="z_t")
                            t2 = tmp.tile([P, NBW], F32, tag="t2")
                            d_t = tmp.tile([P, NBW], F32, tag="d_t")
                            nc.scalar.activation(
                                out=r_t,
                                in_=s_r,
                                func=mybir.ActivationFunctionType.Sigmoid,
                                bias=brz_sb[:, j : j + 1],
                            )
                            nc.scalar.activation(
                                out=z_t,
                                in_=s_z,
                                func=mybir.ActivationFunctionType.Sigmoid,
                                bias=brz_sb[:, KT + j : KT + j + 1],
                            )
                            nc.scalar.activation(
                                out=t2,
                                in_=gh_n,
                                func=mybir.ActivationFunctionType.Identity,
                                bias=bhn_sb[:, j : j + 1],
                            )
                            nc.vector.tensor_tensor(
                                t2, r_t, t2, mybir.AluOpType.mult
                            )
                            nc.vector.tensor_tensor(
                                t2, t2, gi_n, mybir.AluOpType.add
                            )
                            nc.scalar.activation(
                                out=t2,
                                in_=t2,
                                func=mybir.ActivationFunctionType.Tanh,
                                bias=bin_sb[:, j : j + 1],
                            )
                            # h' = n + z*(h - n)
                            nc.vector.tensor_tensor(
                                d_t, h_cur[:, j, cs], t2, mybir.AluOpType.subtract
                            )
                            nc.vector.tensor_tensor(
                                d_t, z_t, d_t, mybir.AluOpType.mult
                            )
                            nc.vector.tensor_tensor(
                                h_new[:, j, cs], t2, d_t, mybir.AluOpType.add
                            )
                    h_cur, h_new = h_new, h_cur

                # h_cur holds the final h for this image (hA after 2 iters)
                nc.sync.dma_start(
                    out=out_d[img].rearrange("k p r -> p k r"), in_=h_cur
                )

    nc.finalize()
    return nc


_NC_CACHE = None


def _get_program():
    global _NC_CACHE
    if _NC_CACHE is None:
        _NC_CACHE = build_program()
    return _NC_CACHE


def _install_ntff_hook():
    """Make trace=True work: register the axon NTFF hook if absent."""
    import types

    try:
        from antenv.axon_hooks import get_axon_ntff_profile_hook  # noqa: F401

        return
    except ImportError:
        pass
    try:
        import antenv
        from trn_agent_boot.trn_boot import _ntff_profile_via_ctypes

        m = types.ModuleType("antenv.axon_hooks")
        m._hook = _ntff_profile_via_ctypes("/opt/axon/libaxon_pjrt.so")
        m.set_axon_ntff_profile_hook = lambda h: setattr(m, "_hook", h)
        m.get_axon_ntff_profile_hook = lambda: m._hook
        sys.modules["antenv.axon_hooks"] = m
        antenv.axon_hooks = m
    except Exception:
        pass


def prepare_inputs(features, boxes, fc_box_w, fc_box_b, fc_input_w, fc_input_b,
                   w_ih, w_hh, b_ih, b_hh):
    """Build the 8 per-core input maps (host-side layout transforms only)."""
    f32 = np.float32
    features = np.asarray(features, f32)
    boxes = np.asarray(boxes, f32)

    # shared (replicated) weight-derived arrays
    w1t = np.ascontiguousarray(
        np.asarray(fc_input_w, f32).T.reshape(KT, P, MT, P).transpose(2, 1, 0, 3)
    )  # [mt, p(k), kt, q(m)]
    bw = np.concatenate(
        [np.asarray(fc_box_w, f32).T, np.asarray(fc_box_b, f32)[None, :]], axis=0
    ).reshape(5, KT, P)
    bw = np.ascontiguousarray(bw)

    def gate_layout(w):
        # w [3F, I] -> w.T [I, 3F] -> [kt, p, gate, jt, q] -> [jt, kt, p, gate*q]
        wt = np.asarray(w, f32).T.reshape(KT, P, 3, KT, P)
        return np.ascontiguousarray(
            wt.transpose(3, 0, 1, 2, 4).reshape(KT, KT, P, 3 * P)
        )

    wih = gate_layout(w_ih)
    whh = gate_layout(w_hh)

    b_ih = np.asarray(b_ih, f32)
    b_hh = np.asarray(b_hh, f32)
    brz = np.ascontiguousarray(
        (b_ih[: 2 * F] + b_hh[: 2 * F]).reshape(2 * KT, P).T
    )  # [p, 2*KT]
    bhn = np.ascontiguousarray(b_hh[2 * F :].reshape(KT, P).T)
    bin_ = np.ascontiguousarray(b_ih[2 * F :].reshape(KT, P).T)
    bi = np.ascontiguousarray(np.asarray(fc_input_b, f32).reshape(MT, P).T)

    in_maps = []
    for c in range(NCORES):
        imgs = slice(c * IMGS, (c + 1) * IMGS)
        h0 = np.ascontiguousarray(
            features[imgs].transpose(0, 2, 1).reshape(IMGS, KT, P, R)
        )
        bx = np.concatenate(
            [
                boxes[imgs].transpose(0, 2, 1),
                np.ones((IMGS, 1, R), f32),
            ],
            axis=1,
        )
        bx = np.ascontiguousarray(bx)
        in_maps.append(
            {
                "h0": h0,
                "bx": bx,
                "bw": bw,
                "w1": w1t,
                "wih": wih,
                "whh": whh,
                "bi": bi,
                "brz": brz,
                "bhn": bhn,
                "bin": bin_,
            }
        )
    return in_maps


def run(in_maps, trace=False):
    nc = _get_program()
    if trace:
        _install_ntff_hook()
    res = run_bass_kernel_spmd(nc, in_maps, list(range(NCORES)), trace=trace)
    return res


def assemble_output(results):
    out = np.empty((B, R, F), np.float32)
    for c in range(NCORES):
        ht = results[c]["out"].reshape(IMGS, F, R)
        for i in range(IMGS):
            out[c * IMGS + i] = ht[i].T
    return out.reshape(B * R, F)


def kernel(**inputs):
    in_maps = prepare_inputs(**inputs)
    res = run(in_maps, trace=False)
    return assemble_output(res.results)


# revision 6
# speedup vs baseline: 1.0388x; 1.0388x over previous
"""GRU message-passing kernel for 8 Trainium2 NeuronCores.

Sharding: data-parallel over the batch dim B=16 -> 2 images per core.
Layout: feature-major (h^T [F, R] per image) so all matmuls take
pre-transposed weights as the stationary operand and activations as the
moving operand -- no on-device transposes. Output transposed on host.
"""

import sys

if "/opt/trn_rl_repo" not in sys.path:
    sys.path.insert(0, "/opt/trn_rl_repo")

import numpy as np

import concourse.bass as bass
import concourse.mybir as mybir
import concourse.tile as tile
from concourse import bacc
from concourse.bass_utils import run_bass_kernel_spmd

B, R, F, I = 16, 1024, 1024, 1024
ITERS = 2
NCORES = 8
IMGS = B // NCORES  # images per core
P = 128
KT = F // P  # 8 k-tiles
MT = I // P  # 8 m-tiles
NB = 2  # column blocks of 512 (PSUM bank limit for fp32)
NBW = R // NB  # 512
INV_DENOM = 1.0 / float(R - 1)

F32 = mybir.dt.float32
F32R = mybir.dt.float32r




def build_program():
    nc = bacc.Bacc("TRN2", target_bir_lowering=False, debug=False, num_devices=NCORES)

    # ---- DRAM tensors (per-core inputs) ----
    # Feature-major initial h (= features^T): [img, kt, p, r]
    h0_d = nc.dram_tensor("h0", [IMGS, KT, P, R], F32R, kind="ExternalInput")
    # boxes^T with an appended ones-row (folds fc_box_b into the matmul):
    bx_d = nc.dram_tensor("bx", [IMGS, 5, R], F32R, kind="ExternalInput")
    # fc_box weights + bias as lhsT rows: [5, jt, q] (row 4 = fc_box_b)
    bw_d = nc.dram_tensor("bw", [5, KT, P], F32R, kind="ExternalInput")
    # fc_input_w^T tiles, per-m-tile contiguous: [mt, p(k), kt, q(m)]
    w1_d = nc.dram_tensor("w1", [MT, P, KT, P], F32R, kind="ExternalInput")
    # GRU weights^T grouped per output f-tile j: [j, kt, p(k), gate(3)*128]
    wih_d = nc.dram_tensor("wih", [KT, KT, P, 3 * P], F32R, kind="ExternalInput")
    whh_d = nc.dram_tensor("whh", [KT, KT, P, 3 * P], F32R, kind="ExternalInput")
    # biases, per-partition layout [p, tile]
    bi_d = nc.dram_tensor("bi", [P, MT], F32, kind="ExternalInput")  # fc_input_b
    brz_d = nc.dram_tensor("brz", [P, 2 * KT], F32, kind="ExternalInput")  # bih+bhh r,z
    bhn_d = nc.dram_tensor("bhn", [P, KT], F32, kind="ExternalInput")  # b_hh n
    bin_d = nc.dram_tensor("bin", [P, KT], F32, kind="ExternalInput")  # b_ih n
    out_d = nc.dram_tensor("out", [IMGS, KT, P, R], F32R, kind="ExternalOutput")

    with tile.TileContext(nc) as tc:
        with (
            tc.tile_pool(name="acts", bufs=1) as acts,
            tc.tile_pool(name="wpool", bufs=2) as wpool,
            tc.tile_pool(name="wg", bufs=4) as wgp,
            tc.tile_pool(name="small", bufs=1) as small,
            tc.tile_pool(name="tmp", bufs=2) as tmp,
            tc.tile_pool(name="stat", bufs=2) as stat,
            tc.tile_pool(name="pbig", bufs=2, space="PSUM") as pbig,
            tc.tile_pool(name="pgate", bufs=2, space="PSUM") as pgate,
        ):
            # persistent activations (per partition: 4 x 32KB = 128KB)
            bufA = acts.tile([P, KT, R], F32R, tag="hA")
            bufB = acts.tile([P, KT, R], F32R, tag="hB")
            bufC = acts.tile([P, KT, R], F32R, tag="aC")
            xS = acts.tile([P, KT, R], F32R, tag="xS")

            # small constants
            bx_sb = small.tile([5, IMGS, R], F32R, tag="bx")
            bw_sb = small.tile([5, KT, P], F32R, tag="bw")
            bi_sb = small.tile([P, MT], F32, tag="bi")
            brz_sb = small.tile([P, 2 * KT], F32, tag="brz")
            bhn_sb = small.tile([P, KT], F32, tag="bhn")
            bin_sb = small.tile([P, KT], F32, tag="bin")
            nc.sync.dma_start(out=bx_sb, in_=bx_d.rearrange("i f r -> f i r"))
            nc.sync.dma_start(out=bw_sb, in_=bw_d[:])
            nc.sync.dma_start(out=bi_sb, in_=bi_d[:])
            nc.sync.dma_start(out=brz_sb, in_=brz_d[:])
            nc.sync.dma_start(out=bhn_sb, in_=bhn_d[:])
            nc.sync.dma_start(out=bin_sb, in_=bin_d[:])

            def load_h0(img, dst):
                # split per k-tile so early f-tiles unblock compute sooner
                for kt in range(KT):
                    nc.sync.dma_start(out=dst[:, kt, :], in_=h0_d[img, kt])

            def store_out(img, srcbuf):
                for kt in range(KT):
                    nc.sync.dma_start(out=out_d[img, kt], in_=srcbuf[:, kt, :])

            def phase_bf_relu(img, h_cur, a_t):
                # a = relu(h * box_feat), box_feat from a K=5 matmul (bias row folded)
                for j in range(KT):
                    bf_ps = pbig.tile([P, R], F32, tag="big")
                    for nb in range(NB):
                        nc.tensor.matmul(
                            bf_ps[:, nb * NBW : (nb + 1) * NBW],
                            bw_sb[:, j, :],
                            bx_sb[:, img, nb * NBW : (nb + 1) * NBW],
                            start=True,
                            stop=True,
                        )
                    nc.vector.tensor_tensor(
                        a_t[:, j, :], h_cur[:, j, :], bf_ps, mybir.AluOpType.mult
                    )
                    nc.scalar.activation(
                        out=a_t[:, j, :],
                        in_=a_t[:, j, :],
                        func=mybir.ActivationFunctionType.Relu,
                    )

            def phase_x_inp(a_t):
                # x^T = W1 @ a with fused row-sum, then inp in place
                s1 = stat.tile([P, MT], F32, tag="s1")
                for mt in range(MT):
                    w1_sb = wpool.tile([P, KT, P], F32R, tag="w1")
                    nc.sync.dma_start(out=w1_sb, in_=w1_d[mt])
                    x_ps = pbig.tile([P, R], F32, tag="big")
                    for k in range(KT):
                        for nb in range(NB):
                            nc.tensor.matmul(
                                x_ps[:, nb * NBW : (nb + 1) * NBW],
                                w1_sb[:, k, :],
                                a_t[:, k, nb * NBW : (nb + 1) * NBW],
                                start=(k == 0),
                                stop=(k == KT - 1),
                            )
                    ssum = stat.tile([P, 1], F32, tag="ssum")
                    nc.scalar.activation(
                        out=xS[:, mt, :],
                        in_=x_ps,
                        func=mybir.ActivationFunctionType.Identity,
                        accum_out=ssum,
                    )
                    nc.scalar.activation(
                        out=s1[:, mt : mt + 1],
                        in_=ssum,
                        func=mybir.ActivationFunctionType.Identity,
                        bias=bi_sb[:, mt : mt + 1],
                        scale=INV_DENOM,
                    )
                    # inp = -x/denom + s1'  (in place, per m-tile)
                    nc.scalar.activation(
                        out=xS[:, mt, :],
                        in_=xS[:, mt, :],
                        func=mybir.ActivationFunctionType.Identity,
                        bias=s1[:, mt : mt + 1],
                        scale=-INV_DENOM,
                    )

            def phase_gates(h_cur, h_new):
                for j in range(KT):
                    # chunked weight tiles: [ih|hh] x [k0-3|k4-7]
                    wtiles = {}
                    for ty, wd in (("ih", wih_d), ("hh", whh_d)):
                        for c in range(2):
                            t = wgp.tile([P, KT // 2, 3 * P], F32R, tag="wg", name=f"wg_{ty}_{c}")
                            nc.sync.dma_start(
                                out=t,
                                in_=wd[j, c * (KT // 2) : (c + 1) * (KT // 2)].rearrange(
                                    "k p c -> p k c"
                                ),
                            )
                            wtiles[(ty, c)] = t

                    def w(ty, k, col):
                        return wtiles[(ty, k // (KT // 2))][
                            :, k % (KT // 2), col * P : (col + 1) * P
                        ]

                    # --- G1: r and z gate sums (ih first, then hh) ---
                    ps = {}
                    for g, tag in ((0, "s_r"), (1, "s_z")):
                        for nb in range(NB):
                            ps[(g, nb)] = pgate.tile([P, NBW], F32, tag=tag, name=f"ps_{tag}_{nb}")
                    for g in (0, 1):
                        for ty, src in (("ih", xS), ("hh", h_cur)):
                            for k in range(KT):
                                for nb in range(NB):
                                    nc.tensor.matmul(
                                        ps[(g, nb)],
                                        w(ty, k, g),
                                        src[:, k, nb * NBW : (nb + 1) * NBW],
                                        start=(ty == "ih" and k == 0),
                                        stop=(ty == "hh" and k == KT - 1),
                                    )
                    r_t = {}
                    z_t = {}
                    for nb in range(NB):
                        r_t[nb] = tmp.tile([P, NBW], F32, tag="r_t", name=f"r_t_{nb}")
                        nc.scalar.activation(
                            out=r_t[nb],
                            in_=ps[(0, nb)],
                            func=mybir.ActivationFunctionType.Sigmoid,
                            bias=brz_sb[:, j : j + 1],
                        )
                        z_t[nb] = tmp.tile([P, NBW], F32, tag="z_t", name=f"z_t_{nb}")
                        nc.scalar.activation(
                            out=z_t[nb],
                            in_=ps[(1, nb)],
                            func=mybir.ActivationFunctionType.Sigmoid,
                            bias=brz_sb[:, KT + j : KT + j + 1],
                        )

                    # --- G2: n-gate inputs (reuse psum slots: ih first) ---
                    gi_n = {}
                    gh_n = {}
                    for nb in range(NB):
                        gi_n[nb] = pgate.tile([P, NBW], F32, tag="s_r", name=f"gi_n_{nb}")
                    for k in range(KT):
                        for nb in range(NB):
                            nc.tensor.matmul(
                                gi_n[nb],
                                w("ih", k, 2),
                                xS[:, k, nb * NBW : (nb + 1) * NBW],
                                start=(k == 0),
                                stop=(k == KT - 1),
                            )
                    for nb in range(NB):
                        gh_n[nb] = pgate.tile([P, NBW], F32, tag="s_z", name=f"gh_n_{nb}")
                    for k in range(KT):
                        for nb in range(NB):
                            nc.tensor.matmul(
                                gh_n[nb],
                                w("hh", k, 2),
                                h_cur[:, k, nb * NBW : (nb + 1) * NBW],
                                start=(k == 0),
                                stop=(k == KT - 1),
                            )

                    # --- elementwise: n = tanh(gi_n + b_in + r*(gh_n + b_hn));
                    #     h' = n + z*(h - n) ---
                    for nb in range(NB):
                        cs = slice(nb * NBW, (nb + 1) * NBW)
                        t2 = tmp.tile([P, NBW], F32, tag="t2")
                        d_t = tmp.tile([P, NBW], F32, tag="d_t")
                        nc.scalar.activation(
                            out=t2,
                            in_=gh_n[nb],
                            func=mybir.ActivationFunctionType.Identity,
                            bias=bhn_sb[:, j : j + 1],
                        )
                        nc.vector.tensor_tensor(t2, r_t[nb], t2, mybir.AluOpType.mult)
                        nc.vector.tensor_tensor(t2, t2, gi_n[nb], mybir.AluOpType.add)
                        nc.scalar.activation(
                            out=t2,
                            in_=t2,
                            func=mybir.ActivationFunctionType.Tanh,
                            bias=bin_sb[:, j : j + 1],
                        )
                        nc.vector.tensor_tensor(
                            d_t, h_cur[:, j, cs], t2, mybir.AluOpType.subtract
                        )
                        nc.vector.tensor_tensor(d_t, z_t[nb], d_t, mybir.AluOpType.mult)
                        nc.vector.tensor_tensor(
                            h_new[:, j, cs], t2, d_t, mybir.AluOpType.add
                        )

            # image 0 uses (A as h0/out, C as a); image 1 rotates (C, A) so its
            # h0 load overlaps image 0's gate phase.
            rot = [(bufA, bufC), (bufC, bufA)]
            for img in range(IMGS):
                hbuf, abuf = rot[img]
                if img == 0:
                    load_h0(0, hbuf)
                chain = [hbuf, bufB, hbuf]  # h buffers per iteration
                for it in range(ITERS):
                    h_cur, h_new = chain[it], chain[it + 1]
                    phase_bf_relu(img, h_cur, abuf)
                    phase_x_inp(abuf)
                    if img + 1 < IMGS and it == ITERS - 1:
                        # prefetch next image's h while this image finishes
                        load_h0(img + 1, rot[img + 1][0])
                    phase_gates(h_cur, h_new)
                store_out(img, chain[-1])

    nc.finalize()
    return nc


_NC_CACHE = None


def _get_program():
    global _NC_CACHE
    if _NC_CACHE is None:
        _NC_CACHE = build_program()
    return _NC_CACHE


def _install_ntff_hook():
    """Make trace=True work: register the axon NTFF hook if absent."""
    import types

    try:
        from antenv.axon_hooks import get_axon_ntff_profile_hook  # noqa: F401

        return
    except ImportError:
        pass
    try:
        import antenv
        from trn_agent_boot.trn_boot import _ntff_profile_via_ctypes

        m = types.ModuleType("antenv.axon_hooks")
        m._hook = _ntff_profile_via_ctypes("/opt/axon/libaxon_pjrt.so")
        m.set_axon_ntff_profile_hook = lambda h: setattr(m, "_hook", h)
        m.get_axon_ntff_profile_hook = lambda: m._hook
        sys.modules["antenv.axon_hooks"] = m
        antenv.axon_hooks = m
    except Exception:
        pass


def prepare_inputs(features, boxes, fc_box_w, fc_box_b, fc_input_w, fc_input_b,
                   w_ih, w_hh, b_ih, b_hh):
    """Build the 8 per-core input maps (host-side layout transforms only)."""
    f32 = np.float32
    features = np.asarray(features, f32)
    boxes = np.asarray(boxes, f32)

    # shared (replicated) weight-derived arrays
    w1t = np.ascontiguousarray(
        np.asarray(fc_input_w, f32).T.reshape(KT, P, MT, P).transpose(2, 1, 0, 3)
    )  # [mt, p(k), kt, q(m)]
    bw = np.concatenate(
        [np.asarray(fc_box_w, f32).T, np.asarray(fc_box_b, f32)[None, :]], axis=0
    ).reshape(5, KT, P)
    bw = np.ascontiguousarray(bw)

    def gate_layout(w):
        # w [3F, I] -> w.T [I, 3F] -> [kt, p, gate, jt, q] -> [jt, kt, p, gate*q]
        wt = np.asarray(w, f32).T.reshape(KT, P, 3, KT, P)
        return np.ascontiguousarray(
            wt.transpose(3, 0, 1, 2, 4).reshape(KT, KT, P, 3 * P)
        )

    wih = gate_layout(w_ih)
    whh = gate_layout(w_hh)

    b_ih = np.asarray(b_ih, f32)
    b_hh = np.asarray(b_hh, f32)
    brz = np.ascontiguousarray(
        (b_ih[: 2 * F] + b_hh[: 2 * F]).reshape(2 * KT, P).T
    )  # [p, 2*KT]
    bhn = np.ascontiguousarray(b_hh[2 * F :].reshape(KT, P).T)
    bin_ = np.ascontiguousarray(b_ih[2 * F :].reshape(KT, P).T)
    bi = np.ascontiguousarray(np.asarray(fc_input_b, f32).reshape(MT, P).T)

    in_maps = []
    for c in range(NCORES):
        imgs = slice(c * IMGS, (c + 1) * IMGS)
        h0 = np.ascontiguousarray(
            features[imgs].transpose(0, 2, 1).reshape(IMGS, KT, P, R)
        )
        bx = np.concatenate(
            [
                boxes[imgs].transpose(0, 2, 1),
                np.ones((IMGS, 1, R), f32),
            ],
            axis=1,
        )
        bx = np.ascontiguousarray(bx)
        in_maps.append(
            {
                "h0": h0,
                "bx": bx,
                "bw": bw,
                "w1": w1t,
                "wih": wih,
                "whh": whh,
                "bi": bi,
                "brz": brz,
                "bhn": bhn,
                "bin": bin_,
            }
        )
    return in_maps


def run(in_maps, trace=False):
    nc = _get_program()
    if trace:
        _install_ntff_hook()
    res = run_bass_kernel_spmd(nc, in_maps, list(range(NCORES)), trace=trace)
    return res


def assemble_output(results):
    out = np.empty((B, R, F), np.float32)
    for c in range(NCORES):
        ht = results[c]["out"].reshape(IMGS, F, R)
        for i in range(IMGS):
            out[c * IMGS + i] = ht[i].T
    return out.reshape(B * R, F)


def kernel(**inputs):
    in_maps = prepare_inputs(**inputs)
    res = run(in_maps, trace=False)
    return assemble_output(res.results)


# revision 7
# speedup vs baseline: 1.1391x; 1.0965x over previous
"""GRU message-passing kernel for 8 Trainium2 NeuronCores.

Sharding: data-parallel over the batch dim B=16 -> 2 images per core.
Layout: feature-major (h^T [F, R] per image) so all matmuls take
pre-transposed weights as the stationary operand and activations as the
moving operand -- no on-device transposes. Output transposed on host.
"""

import sys

if "/opt/trn_rl_repo" not in sys.path:
    sys.path.insert(0, "/opt/trn_rl_repo")

import numpy as np

import concourse.bass as bass
import concourse.mybir as mybir
import concourse.tile as tile
from concourse import bacc
from concourse.bass_utils import run_bass_kernel_spmd

B, R, F, I = 16, 1024, 1024, 1024
ITERS = 2
NCORES = 8
IMGS = B // NCORES  # images per core
P = 128
KT = F // P  # 8 k-tiles
MT = I // P  # 8 m-tiles
NB = 2  # column blocks of 512 (PSUM bank limit for fp32)
NBW = R // NB  # 512
INV_DENOM = 1.0 / float(R - 1)

F32 = mybir.dt.float32
F32R = mybir.dt.float32r
F16 = mybir.dt.float16




def build_program():
    nc = bacc.Bacc("TRN2", target_bir_lowering=False, debug=False, num_devices=NCORES)

    # ---- DRAM tensors (per-core inputs) ----
    # Feature-major initial h (= features^T): [img, kt, p, r]
    h0_d = nc.dram_tensor("h0", [IMGS, KT, P, R], F16, kind="ExternalInput")
    # boxes^T with an appended ones-row (folds fc_box_b into the matmul):
    bx_d = nc.dram_tensor("bx", [IMGS, 5, R], F16, kind="ExternalInput")
    # fc_box weights + bias as lhsT rows: [5, jt, q] (row 4 = fc_box_b)
    bw_d = nc.dram_tensor("bw", [5, KT, P], F16, kind="ExternalInput")
    # fc_input_w^T tiles, per-m-tile contiguous: [mt, p(k), kt, q(m)]
    w1_d = nc.dram_tensor("w1", [MT, P, KT, P], F16, kind="ExternalInput")
    # GRU weights^T grouped per output f-tile j: [j, kt, p(k), gate(3)*128]
    wih_d = nc.dram_tensor("wih", [KT, KT, P, 3 * P], F16, kind="ExternalInput")
    whh_d = nc.dram_tensor("whh", [KT, KT, P, 3 * P], F16, kind="ExternalInput")
    # biases, per-partition layout [p, tile]
    bi_d = nc.dram_tensor("bi", [P, MT], F32, kind="ExternalInput")  # fc_input_b
    brz_d = nc.dram_tensor("brz", [P, 2 * KT], F32, kind="ExternalInput")  # bih+bhh r,z
    bhn_d = nc.dram_tensor("bhn", [P, KT], F32, kind="ExternalInput")  # b_hh n
    bin_d = nc.dram_tensor("bin", [P, KT], F32, kind="ExternalInput")  # b_ih n
    out_d = nc.dram_tensor("out", [IMGS, KT, P, R], F16, kind="ExternalOutput")

    with tile.TileContext(nc) as tc:
        with (
            tc.tile_pool(name="acts", bufs=1) as acts,
            tc.tile_pool(name="wpool", bufs=2) as wpool,
            tc.tile_pool(name="wg", bufs=4) as wgp,
            tc.tile_pool(name="small", bufs=1) as small,
            tc.tile_pool(name="tmp", bufs=2) as tmp,
            tc.tile_pool(name="stat", bufs=2) as stat,
            tc.tile_pool(name="pbig", bufs=2, space="PSUM") as pbig,
            tc.tile_pool(name="pgate", bufs=2, space="PSUM") as pgate,
        ):
            # persistent activations (per partition: 4 x 32KB = 128KB)
            bufA = acts.tile([P, KT, R], F16, tag="hA")
            bufB = acts.tile([P, KT, R], F16, tag="hB")
            bufC = acts.tile([P, KT, R], F16, tag="aC")
            xS = acts.tile([P, KT, R], F16, tag="xS")

            # small constants
            bx_sb = small.tile([5, IMGS, R], F16, tag="bx")
            bw_sb = small.tile([5, KT, P], F16, tag="bw")
            bi_sb = small.tile([P, MT], F32, tag="bi")
            brz_sb = small.tile([P, 2 * KT], F32, tag="brz")
            bhn_sb = small.tile([P, KT], F32, tag="bhn")
            bin_sb = small.tile([P, KT], F32, tag="bin")
            nc.sync.dma_start(out=bx_sb, in_=bx_d.rearrange("i f r -> f i r"))
            nc.sync.dma_start(out=bw_sb, in_=bw_d[:])
            nc.sync.dma_start(out=bi_sb, in_=bi_d[:])
            nc.sync.dma_start(out=brz_sb, in_=brz_d[:])
            nc.sync.dma_start(out=bhn_sb, in_=bhn_d[:])
            nc.sync.dma_start(out=bin_sb, in_=bin_d[:])

            def load_h0(img, dst):
                # split per k-tile so early f-tiles unblock compute sooner
                for kt in range(KT):
                    nc.sync.dma_start(out=dst[:, kt, :], in_=h0_d[img, kt])

            def store_out(img, srcbuf):
                for kt in range(KT):
                    nc.sync.dma_start(out=out_d[img, kt], in_=srcbuf[:, kt, :])

            def phase_bf_relu(img, h_cur, a_t):
                # a = relu(h * box_feat), box_feat from a K=5 matmul (bias row folded)
                for j in range(KT):
                    bf_ps = pbig.tile([P, R], F32, tag="big")
                    for nb in range(NB):
                        nc.tensor.matmul(
                            bf_ps[:, nb * NBW : (nb + 1) * NBW],
                            bw_sb[:, j, :],
                            bx_sb[:, img, nb * NBW : (nb + 1) * NBW],
                            start=True,
                            stop=True,
                        )
                    nc.vector.tensor_tensor(
                        a_t[:, j, :], h_cur[:, j, :], bf_ps, mybir.AluOpType.mult
                    )
                    nc.scalar.activation(
                        out=a_t[:, j, :],
                        in_=a_t[:, j, :],
                        func=mybir.ActivationFunctionType.Relu,
                    )

            def phase_x_inp(a_t):
                # x^T = W1 @ a with fused row-sum, then inp in place
                s1 = stat.tile([P, MT], F32, tag="s1")
                for mt in range(MT):
                    w1_sb = wpool.tile([P, KT, P], F16, tag="w1")
                    nc.sync.dma_start(out=w1_sb, in_=w1_d[mt])
                    x_ps = pbig.tile([P, R], F32, tag="big")
                    for k in range(KT):
                        for nb in range(NB):
                            nc.tensor.matmul(
                                x_ps[:, nb * NBW : (nb + 1) * NBW],
                                w1_sb[:, k, :],
                                a_t[:, k, nb * NBW : (nb + 1) * NBW],
                                start=(k == 0),
                                stop=(k == KT - 1),
                            )
                    ssum = stat.tile([P, 1], F32, tag="ssum")
                    nc.scalar.activation(
                        out=xS[:, mt, :],
                        in_=x_ps,
                        func=mybir.ActivationFunctionType.Identity,
                        accum_out=ssum,
                    )
                    nc.scalar.activation(
                        out=s1[:, mt : mt + 1],
                        in_=ssum,
                        func=mybir.ActivationFunctionType.Identity,
                        bias=bi_sb[:, mt : mt + 1],
                        scale=INV_DENOM,
                    )
                    # inp = -x/denom + s1'  (in place, per m-tile)
                    nc.scalar.activation(
                        out=xS[:, mt, :],
                        in_=xS[:, mt, :],
                        func=mybir.ActivationFunctionType.Identity,
                        bias=s1[:, mt : mt + 1],
                        scale=-INV_DENOM,
                    )

            def phase_gates(h_cur, h_new):
                for j in range(KT):
                    # chunked weight tiles: [ih|hh] x [k0-3|k4-7]
                    wtiles = {}
                    for ty, wd in (("ih", wih_d), ("hh", whh_d)):
                        for c in range(2):
                            t = wgp.tile([P, KT // 2, 3 * P], F16, tag="wg", name=f"wg_{ty}_{c}")
                            nc.sync.dma_start(
                                out=t,
                                in_=wd[j, c * (KT // 2) : (c + 1) * (KT // 2)].rearrange(
                                    "k p c -> p k c"
                                ),
                            )
                            wtiles[(ty, c)] = t

                    def w(ty, k, col):
                        return wtiles[(ty, k // (KT // 2))][
                            :, k % (KT // 2), col * P : (col + 1) * P
                        ]

                    # --- G1: r and z gate sums (ih first, then hh) ---
                    ps = {}
                    for g, tag in ((0, "s_r"), (1, "s_z")):
                        for nb in range(NB):
                            ps[(g, nb)] = pgate.tile([P, NBW], F32, tag=tag, name=f"ps_{tag}_{nb}")
                    for g in (0, 1):
                        for ty, src in (("ih", xS), ("hh", h_cur)):
                            for k in range(KT):
                                for nb in range(NB):
                                    nc.tensor.matmul(
                                        ps[(g, nb)],
                                        w(ty, k, g),
                                        src[:, k, nb * NBW : (nb + 1) * NBW],
                                        start=(ty == "ih" and k == 0),
                                        stop=(ty == "hh" and k == KT - 1),
                                    )
                    r_t = {}
                    z_t = {}
                    for nb in range(NB):
                        r_t[nb] = tmp.tile([P, NBW], F32, tag="r_t", name=f"r_t_{nb}")
                        nc.scalar.activation(
                            out=r_t[nb],
                            in_=ps[(0, nb)],
                            func=mybir.ActivationFunctionType.Sigmoid,
                            bias=brz_sb[:, j : j + 1],
                        )
                        z_t[nb] = tmp.tile([P, NBW], F32, tag="z_t", name=f"z_t_{nb}")
                        nc.scalar.activation(
                            out=z_t[nb],
                            in_=ps[(1, nb)],
                            func=mybir.ActivationFunctionType.Sigmoid,
                            bias=brz_sb[:, KT + j : KT + j + 1],
                        )

                    # --- G2: n-gate inputs (reuse psum slots: ih first) ---
                    gi_n = {}
                    gh_n = {}
                    for nb in range(NB):
                        gi_n[nb] = pgate.tile([P, NBW], F32, tag="s_r", name=f"gi_n_{nb}")
                    for k in range(KT):
                        for nb in range(NB):
                            nc.tensor.matmul(
                                gi_n[nb],
                                w("ih", k, 2),
                                xS[:, k, nb * NBW : (nb + 1) * NBW],
                                start=(k == 0),
                                stop=(k == KT - 1),
                            )
                    for nb in range(NB):
                        gh_n[nb] = pgate.tile([P, NBW], F32, tag="s_z", name=f"gh_n_{nb}")
                    for k in range(KT):
                        for nb in range(NB):
                            nc.tensor.matmul(
                                gh_n[nb],
                                w("hh", k, 2),
                                h_cur[:, k, nb * NBW : (nb + 1) * NBW],
                                start=(k == 0),
                                stop=(k == KT - 1),
                            )

                    # --- elementwise: n = tanh(gi_n + b_in + r*(gh_n + b_hn));
                    #     h' = n + z*(h - n) ---
                    for nb in range(NB):
                        cs = slice(nb * NBW, (nb + 1) * NBW)
                        t2 = tmp.tile([P, NBW], F32, tag="t2")
                        d_t = tmp.tile([P, NBW], F32, tag="d_t")
                        nc.scalar.activation(
                            out=t2,
                            in_=gh_n[nb],
                            func=mybir.ActivationFunctionType.Identity,
                            bias=bhn_sb[:, j : j + 1],
                        )
                        nc.vector.tensor_tensor(t2, r_t[nb], t2, mybir.AluOpType.mult)
                        nc.vector.tensor_tensor(t2, t2, gi_n[nb], mybir.AluOpType.add)
                        nc.scalar.activation(
                            out=t2,
                            in_=t2,
                            func=mybir.ActivationFunctionType.Tanh,
                            bias=bin_sb[:, j : j + 1],
                        )
                        nc.vector.tensor_tensor(
                            d_t, h_cur[:, j, cs], t2, mybir.AluOpType.subtract
                        )
                        nc.vector.tensor_tensor(d_t, z_t[nb], d_t, mybir.AluOpType.mult)
                        nc.vector.tensor_tensor(
                            h_new[:, j, cs], t2, d_t, mybir.AluOpType.add
                        )

            # image 0 uses (A as h0/out, C as a); image 1 rotates (C, A) so its
            # h0 load overlaps image 0's gate phase.
            rot = [(bufA, bufC), (bufC, bufA)]
            for img in range(IMGS):
                hbuf, abuf = rot[img]
                if img == 0:
                    load_h0(0, hbuf)
                chain = [hbuf, bufB, hbuf]  # h buffers per iteration
                for it in range(ITERS):
                    h_cur, h_new = chain[it], chain[it + 1]
                    phase_bf_relu(img, h_cur, abuf)
                    phase_x_inp(abuf)
                    if img + 1 < IMGS and it == ITERS - 1:
                        # prefetch next image's h while this image finishes
                        load_h0(img + 1, rot[img + 1][0])
                    phase_gates(h_cur, h_new)
                store_out(img, chain[-1])

    nc.finalize()
    return nc


_NC_CACHE = None


def _get_program():
    global _NC_CACHE
    if _NC_CACHE is None:
        _NC_CACHE = build_program()
    return _NC_CACHE


def _install_ntff_hook():
    """Make trace=True work: register the axon NTFF hook if absent."""
    import types

    try:
        from antenv.axon_hooks import get_axon_ntff_profile_hook  # noqa: F401

        return
    except ImportError:
        pass
    try:
        import antenv
        from trn_agent_boot.trn_boot import _ntff_profile_via_ctypes

        m = types.ModuleType("antenv.axon_hooks")
        m._hook = _ntff_profile_via_ctypes("/opt/axon/libaxon_pjrt.so")
        m.set_axon_ntff_profile_hook = lambda h: setattr(m, "_hook", h)
        m.get_axon_ntff_profile_hook = lambda: m._hook
        sys.modules["antenv.axon_hooks"] = m
        antenv.axon_hooks = m
    except Exception:
        pass


def prepare_inputs(features, boxes, fc_box_w, fc_box_b, fc_input_w, fc_input_b,
                   w_ih, w_hh, b_ih, b_hh):
    """Build the 8 per-core input maps (host-side layout transforms only)."""
    f32 = np.float32
    f16 = np.float16
    features = np.asarray(features, f32)
    boxes = np.asarray(boxes, f32)

    # shared (replicated) weight-derived arrays
    w1t = np.ascontiguousarray(
        np.asarray(fc_input_w, f32).T.reshape(KT, P, MT, P).transpose(2, 1, 0, 3)
    )  # [mt, p(k), kt, q(m)]
    bw = np.concatenate(
        [np.asarray(fc_box_w, f32).T, np.asarray(fc_box_b, f32)[None, :]], axis=0
    ).reshape(5, KT, P)
    bw = np.ascontiguousarray(bw)

    def gate_layout(w):
        # w [3F, I] -> w.T [I, 3F] -> [kt, p, gate, jt, q] -> [jt, kt, p, gate*q]
        wt = np.asarray(w, f32).T.reshape(KT, P, 3, KT, P)
        return np.ascontiguousarray(
            wt.transpose(3, 0, 1, 2, 4).reshape(KT, KT, P, 3 * P)
        )

    wih = gate_layout(w_ih).astype(f16)
    whh = gate_layout(w_hh).astype(f16)

    b_ih = np.asarray(b_ih, f32)
    b_hh = np.asarray(b_hh, f32)
    brz = np.ascontiguousarray(
        (b_ih[: 2 * F] + b_hh[: 2 * F]).reshape(2 * KT, P).T
    )  # [p, 2*KT]
    bhn = np.ascontiguousarray(b_hh[2 * F :].reshape(KT, P).T)
    bin_ = np.ascontiguousarray(b_ih[2 * F :].reshape(KT, P).T)
    bi = np.ascontiguousarray(np.asarray(fc_input_b, f32).reshape(MT, P).T)

    in_maps = []
    for c in range(NCORES):
        imgs = slice(c * IMGS, (c + 1) * IMGS)
        h0 = np.ascontiguousarray(
            features[imgs].transpose(0, 2, 1).reshape(IMGS, KT, P, R)
        )
        bx = np.concatenate(
            [
                boxes[imgs].transpose(0, 2, 1),
                np.ones((IMGS, 1, R), f32),
            ],
            axis=1,
        )
        bx = np.ascontiguousarray(bx)
        in_maps.append(
            {
                "h0": h0.astype(f16),
                "bx": bx.astype(f16),
                "bw": bw.astype(f16),
                "w1": w1t.astype(f16),
                "wih": wih,
                "whh": whh,
                "bi": bi,
                "brz": brz,
                "bhn": bhn,
                "bin": bin_,
            }
        )
    return in_maps


def run(in_maps, trace=False):
    nc = _get_program()
    if trace:
        _install_ntff_hook()
    res = run_bass_kernel_spmd(nc, in_maps, list(range(NCORES)), trace=trace)
    return res


def assemble_output(results):
    out = np.empty((B, R, F), np.float32)
    for c in range(NCORES):
        ht = results[c]["out"].astype(np.float32).reshape(IMGS, F, R)
        for i in range(IMGS):
            out[c * IMGS + i] = ht[i].T
    return out.reshape(B * R, F)


def kernel(**inputs):
    in_maps = prepare_inputs(**inputs)
    res = run(in_maps, trace=False)
    return assemble_output(res.results)
